# revision 1
# baseline (speedup 1.0000x reference)
"""GRU cell kernel for Trainium2, data-parallel over 8 NeuronCores.

Reference computation (B=4096, I=H=2048, C=I+H=4096):
    combined   = [x, h]                                   [B, C]
    to_update  = sigmoid(combined @ W_update.T + b_u)     [B, H]
    to_select  = sigmoid(combined @ W_select.T + b_s)     [B, H]
    updated    = h * to_update
    new_comb   = [x, updated]
    predictions= tanh(new_comb @ W_predict.T + b_p)
    h_new      = h * (1 - to_select) + predictions * to_select

Sharding: batch split 8 ways (512 rows/core), weights replicated.
On-chip layout is [feature, batch] (transposed), so each weight tile
[128c, 128h] is the stationary matmul operand and activation tiles
[128c, 512b] are the moving operand -- no on-chip transposes anywhere.
Matmuls run in bf16 (inputs host-cast) with fp32 PSUM accumulation;
gates and the final blend stay fp32.
"""

from contextlib import ExitStack

import numpy as np
import ml_dtypes

import concourse.bass as bass
import concourse.tile as tile
import concourse.mybir as mybir
from concourse import bacc
from concourse.bass_utils import run_bass_kernel_spmd

BF16 = mybir.dt.bfloat16
F32 = mybir.dt.float32
NPBF16 = ml_dtypes.bfloat16

B, I, H = 4096, 2048, 2048
C = I + H
NCORES = 8
BS = B // NCORES            # 512 batch rows per core
P = 128                     # SBUF partitions
HT = H // P                 # 16 output-row tiles
IT = I // P                 # 16 x feature tiles
CT = C // P                 # 32 contraction tiles
ACT_F = mybir.ActivationFunctionType

_PROGRAM = None


def _build_program():
    nc = bacc.Bacc("TRN2")

    xT = nc.dram_tensor("xT", [IT, P, BS], BF16, kind="ExternalInput")
    hT32 = nc.dram_tensor("hT32", [HT, P, BS], F32, kind="ExternalInput")
    Wu = nc.dram_tensor("Wu", [HT, P, C], BF16, kind="ExternalInput")
    Ws = nc.dram_tensor("Ws", [HT, P, C], BF16, kind="ExternalInput")
    Wp = nc.dram_tensor("Wp", [HT, P, C], BF16, kind="ExternalInput")
    bu = nc.dram_tensor("bu", [P, HT], F32, kind="ExternalInput")
    bsel = nc.dram_tensor("bsel", [P, HT], F32, kind="ExternalInput")
    bp = nc.dram_tensor("bp", [P, HT], F32, kind="ExternalInput")
    out = nc.dram_tensor("out", [HT, P, BS], F32, kind="ExternalOutput")

    with tile.TileContext(nc) as tc, ExitStack() as ctx:
        singles = ctx.enter_context(tc.tile_pool(name="singles", bufs=1))
        wpool = ctx.enter_context(tc.tile_pool(name="wpool", bufs=4))
        pspool = ctx.enter_context(tc.tile_pool(name="ps", bufs=8, space="PSUM"))
        work = ctx.enter_context(tc.tile_pool(name="work", bufs=4))

        bu_sb = singles.tile([P, HT], F32, name="bu_sb")
        nc.sync.dma_start(bu_sb[:], bu[:])
        bs_sb = singles.tile([P, HT], F32, name="bs_sb")
        nc.sync.dma_start(bs_sb[:], bsel[:])
        bp_sb = singles.tile([P, HT], F32, name="bp_sb")
        nc.sync.dma_start(bp_sb[:], bp[:])

        # combined.T tiles: 16 x-tiles then 16 h-tiles (all bf16 [128, 512])
        comb = []
        for n in range(IT):
            t = singles.tile([P, BS], BF16, name=f"combx{n}", tag=f"combx{n}")
            nc.sync.dma_start(t[:], xT[n])
            comb.append(t)
        # h arrives fp32 (needed for the final blend); bf16 copies are cast
        # on-chip to avoid a second HBM upload of h.
        h32 = []
        for i in range(HT):
            t = singles.tile([P, BS], F32, name=f"h32_{i}", tag=f"h32_{i}")
            nc.sync.dma_start(t[:], hT32[i])
            h32.append(t)
        for i in range(HT):
            t = singles.tile([P, BS], BF16, name=f"combh{i}", tag=f"combh{i}")
            nc.vector.tensor_copy(t[:], h32[i][:])
            comb.append(t)

        upd = [
            singles.tile([P, BS], BF16, name=f"upd{i}", tag=f"upd{i}")
            for i in range(HT)
        ]
        sel = [
            singles.tile([P, BS], F32, name=f"sel{i}", tag=f"sel{i}")
            for i in range(HT)
        ]

        def gemm(W, rhs_tiles, i):
            """psum[128h, 512b] = sum_c W_tile[i].T @ rhs  (bf16, fp32 accum)"""
            wblk = wpool.tile([P, C], BF16, tag="wblk", name="wblk")
            half = C // 2
            nc.sync.dma_start(wblk[:, 0:half], W[i, :, 0:half])
            nc.sync.dma_start(wblk[:, half:C], W[i, :, half:C])
            ps = pspool.tile([P, BS], F32, tag="ps", name="ps")
            for n in range(CT):
                nc.tensor.matmul(
                    ps,
                    wblk[:, n * P:(n + 1) * P],
                    rhs_tiles[n],
                    start=(n == 0),
                    stop=(n == CT - 1),
                )
            return ps

        # update gate -> updated = h * sigmoid(z_u)  (bf16, feeds matmul 3)
        for i in range(HT):
            ps = gemm(Wu, comb, i)
            u = work.tile([P, BS], BF16, tag="u", name="u")
            nc.scalar.activation(u[:], ps[:], ACT_F.Sigmoid, bias=bu_sb[:, i:i + 1])
            nc.vector.tensor_mul(upd[i][:], comb[IT + i][:], u[:])

        # select gate (fp32, used in final blend)
        for i in range(HT):
            ps = gemm(Ws, comb, i)
            nc.scalar.activation(
                sel[i][:], ps[:], ACT_F.Sigmoid, bias=bs_sb[:, i:i + 1]
            )

        # predictions + blend: h_new = h + sel * (tanh(z_p) - h)
        newcomb = comb[:IT] + upd
        for i in range(HT):
            ps = gemm(Wp, newcomb, i)
            p_t = work.tile([P, BS], F32, tag="p", name="p_t")
            nc.scalar.activation(p_t[:], ps[:], ACT_F.Tanh, bias=bp_sb[:, i:i + 1])
            d = work.tile([P, BS], F32, tag="d", name="d")
            nc.vector.tensor_sub(d[:], p_t[:], h32[i][:])
            nc.vector.tensor_mul(d[:], d[:], sel[i][:])
            o = work.tile([P, BS], F32, tag="o", name="o")
            nc.vector.tensor_add(o[:], h32[i][:], d[:])
            nc.sync.dma_start(out[i], o[:])

    nc.finalize()
    return nc


def _get_program():
    global _PROGRAM
    if _PROGRAM is None:
        _PROGRAM = _build_program()
    return _PROGRAM


def _pack_weight(w):
    """[H, C] fp32 -> [HT, P, C] bf16 with [i, p, n*128+m] = W[i*128+m, n*128+p].

    Slice [i] is then an SBUF block whose column window n*128:(n+1)*128 is the
    stationary operand (lhsT = W.T tile) for contraction tile n.
    """
    wb = np.asarray(w, dtype=np.float32).astype(NPBF16)
    return np.ascontiguousarray(
        wb.reshape(HT, P, CT, P).transpose(0, 3, 2, 1).reshape(HT, P, C)
    )


def _prep_inputs(x, h, W_update, b_update, W_select, b_select, W_predict, b_predict):
    x = np.asarray(x, dtype=np.float32)
    h = np.asarray(h, dtype=np.float32)

    Wu = _pack_weight(W_update)
    Ws = _pack_weight(W_select)
    Wp = _pack_weight(W_predict)
    bu = np.ascontiguousarray(
        np.asarray(b_update, dtype=np.float32).reshape(HT, P).T
    )
    bsel = np.ascontiguousarray(
        np.asarray(b_select, dtype=np.float32).reshape(HT, P).T
    )
    bp = np.ascontiguousarray(
        np.asarray(b_predict, dtype=np.float32).reshape(HT, P).T
    )

    in_maps = []
    for c in range(NCORES):
        rows = slice(c * BS, (c + 1) * BS)
        xT = np.ascontiguousarray(x[rows].T.astype(NPBF16).reshape(IT, P, BS))
        hT32 = np.ascontiguousarray(h[rows].T.reshape(HT, P, BS))
        in_maps.append(
            {
                "xT": xT,
                "hT32": hT32,
                "Wu": Wu,
                "Ws": Ws,
                "Wp": Wp,
                "bu": bu,
                "bsel": bsel,
                "bp": bp,
            }
        )
    return in_maps


def kernel(x, h, W_update, b_update, W_select, b_select, W_predict, b_predict,
           _trace=False):
    nc = _get_program()
    in_maps = _prep_inputs(
        x, h, W_update, b_update, W_select, b_select, W_predict, b_predict
    )
    res = run_bass_kernel_spmd(
        nc, in_maps, core_ids=list(range(NCORES)), trace=_trace
    )
    h_new = np.empty((B, H), dtype=np.float32)
    for c in range(NCORES):
        rows = slice(c * BS, (c + 1) * BS)
        h_new[rows] = res.results[c]["out"].reshape(H, BS).T
    if _trace:
        return h_new, res
    return h_new



# revision 2
# speedup vs baseline: 1.2552x; 1.2552x over previous
"""GRU cell kernel for Trainium2, data-parallel over 8 NeuronCores.

Reference computation (B=4096, I=H=2048, C=I+H=4096):
    combined   = [x, h]                                   [B, C]
    to_update  = sigmoid(combined @ W_update.T + b_u)     [B, H]
    to_select  = sigmoid(combined @ W_select.T + b_s)     [B, H]
    updated    = h * to_update
    new_comb   = [x, updated]
    predictions= tanh(new_comb @ W_predict.T + b_p)
    h_new      = h * (1 - to_select) + predictions * to_select
    = h + to_select * (predictions - h)

Sharding: batch split 8 ways (512 rows/core), weights replicated.
On-chip layout is [feature, batch] (transposed); weight tiles are the
stationary matmul operand, activation tiles [128c, 512b] the moving one.

Precision: the update gate runs in fp8e4 with DoubleRow perf mode
(2 contraction rows/cycle, ~1.4x the bf16 matmul rate); its quantization
error is damped by sigmoid' and the downstream contraction, landing at
~6e-3 rel err overall (vs 3.7e-3 all-bf16, gate 2e-2). The select gate
is the error-critical one (its error multiplies (p - h)) and stays bf16,
as does predict. Weights for the fp8 gate are pre-scaled by 64 on the
host so they sit in e4m3's normal range; the sigmoid's input scale
folds the 1/64 back out. h is used in bf16 everywhere (no fp32 h
upload); psum accumulation is fp32 throughout.

Schedule: dummy matmuls on a memset tile warm the PE clock (HAM) while
the head DMAs land; the fp8 gate goes first because its input bytes are
half (2 MB fp8 combined + 0.5 MB/block weights), minimizing the time to
first full accumulation group. bf16 activations stream in behind the
fp8 weight blocks during the update phase. The last predict tile is
split into two half-batch groups so the activation/blend/store chain of
the first half hides under the second half's matmuls.
"""

from contextlib import ExitStack

import numpy as np
import ml_dtypes

import concourse.bass as bass  # noqa: F401  (kept for parity with docs)
import concourse.tile as tile
import concourse.mybir as mybir
from concourse import bacc
from concourse.bass_utils import run_bass_kernel_spmd

BF16 = mybir.dt.bfloat16
F8 = mybir.dt.float8e4
F32 = mybir.dt.float32
NPBF16 = ml_dtypes.bfloat16
NPF8 = ml_dtypes.float8_e4m3

B, I, H = 4096, 2048, 2048
C = I + H
NCORES = 8
BS = B // NCORES            # 512 batch rows per core
P = 128                     # SBUF partitions
HT = H // P                 # 16 output-row tiles
IT = I // P                 # 16 x feature tiles
CT = C // P                 # 32 contraction tiles
WSCALE = 64.0               # fp8 weight pre-scale (W*64 ~ N(0,1))
NWARM = 12                  # PE-clock warm-up matmuls
ACT_F = mybir.ActivationFunctionType
DR = mybir.MatmulPerfMode.DoubleRow

_PROGRAM = None


def _build_program():
    nc = bacc.Bacc("TRN2")

    xh8 = nc.dram_tensor("xh8", [P, CT, BS], F8, kind="ExternalInput")
    xhb = nc.dram_tensor("xhb", [P, CT, BS], BF16, kind="ExternalInput")
    Wu8 = nc.dram_tensor("Wu8", [HT, P, CT, P], F8, kind="ExternalInput")
    Ws = nc.dram_tensor("Ws", [HT, P, CT, P], BF16, kind="ExternalInput")
    Wp = nc.dram_tensor("Wp", [HT, P, CT, P], BF16, kind="ExternalInput")
    bias = nc.dram_tensor("bias", [P, 3 * HT], F32, kind="ExternalInput")
    out = nc.dram_tensor("out", [HT, P, BS], F32, kind="ExternalOutput")

    with tile.TileContext(nc) as tc, ExitStack() as ctx:
        singles = ctx.enter_context(tc.tile_pool(name="singles", bufs=1))
        wpool = ctx.enter_context(tc.tile_pool(name="wpool", bufs=3))
        pspool = ctx.enter_context(tc.tile_pool(name="ps", bufs=8, space="PSUM"))
        work = ctx.enter_context(tc.tile_pool(name="work", bufs=4))

        # Dummy matmuls on a zeroed tile: keeps the PE HAM at full clock
        # while the first input DMAs are still in flight.
        warm = singles.tile([P, BS], BF16, name="warm")
        nc.vector.memset(warm[:], 0.0)
        for _ in range(NWARM):
            wps = pspool.tile([P, BS], F32, tag="ps", name="wps")
            nc.tensor.matmul(wps, warm[:, 0:P], warm[:], start=True, stop=True)

        bias_sb = singles.tile([P, 3 * HT], F32, name="bias_sb")
        nc.sync.dma_start(bias_sb[:], bias[:])

        comb8 = singles.tile([P, CT, BS], F8, name="comb8")
        combb = singles.tile([P, CT, BS], BF16, name="combb")
        newc = singles.tile([P, HT, BS], BF16, name="newc")
        usb = singles.tile([P, HT, BS], BF16, name="usb")
        sel = singles.tile([P, HT, BS], F32, name="sel")

        # Head DMAs: first fp8 weight block, then the fp8 activations.
        w8_0 = wpool.tile([P, CT, P], F8, tag="w8", name="w8_0")
        nc.sync.dma_start(w8_0[:], Wu8[0])
        for c in range(4):
            nc.sync.dma_start(
                comb8[:, 8 * c:8 * (c + 1), :], xh8[:, 8 * c:8 * (c + 1), :]
            )

        # ---- update gate, fp8 DoubleRow ----
        for i in range(HT):
            if i == 0:
                w8 = w8_0
            else:
                w8 = wpool.tile([P, CT, P], F8, tag="w8", name="w8")
                nc.sync.dma_start(w8[:], Wu8[i])
            ps = pspool.tile([P, BS], F32, tag="ps", name="ps")
            for n in range(CT // 2):
                nc.tensor.matmul(
                    ps,
                    w8[:, 2 * n:2 * n + 2, :],
                    comb8[:, 2 * n:2 * n + 2, :],
                    start=(n == 0),
                    stop=(n == CT // 2 - 1),
                    perf_mode=DR,
                )
            nc.scalar.activation(
                usb[:, i:i + 1, :], ps[:], ACT_F.Sigmoid,
                bias=bias_sb[:, i:i + 1], scale=1.0 / WSCALE,
            )
            # bf16 activations stream in behind the fp8 weight blocks
            if i in (2, 4, 6, 8):
                c = i // 2 - 1
                nc.sync.dma_start(
                    combb[:, 8 * c:8 * (c + 1), :], xhb[:, 8 * c:8 * (c + 1), :]
                )

        # updated = h * u  (fires when the bf16 h tiles land; feeds predict)
        for i in range(HT):
            nc.vector.tensor_mul(
                newc[:, i:i + 1, :], combb[:, HT + i:HT + i + 1, :],
                usb[:, i:i + 1, :],
            )

        # ---- select gate, bf16; sel kept fp32 for the blend ----
        for i in range(HT):
            wb = wpool.tile([P, CT, P], BF16, tag="wb", name="wb")
            nc.sync.dma_start(wb[:], Ws[i])
            ps = pspool.tile([P, BS], F32, tag="ps", name="ps")
            for n in range(CT):
                nc.tensor.matmul(
                    ps, wb[:, n:n + 1, :], combb[:, n:n + 1, :],
                    start=(n == 0), stop=(n == CT - 1),
                )
            nc.scalar.activation(
                sel[:, i:i + 1, :], ps[:], ACT_F.Sigmoid,
                bias=bias_sb[:, HT + i:HT + i + 1],
            )

        # ---- predict gate + blend ----
        def pchain(i, c0, c1, ps):
            w = c1 - c0
            p_t = work.tile([P, BS], F32, tag="p", name="p_t")
            nc.scalar.activation(
                p_t[:, 0:w], ps[:, 0:w], ACT_F.Tanh,
                bias=bias_sb[:, 2 * HT + i:2 * HT + i + 1],
            )
            d = work.tile([P, BS], F32, tag="d", name="d")
            nc.vector.tensor_sub(
                d[:, 0:w], p_t[:, 0:w], combb[:, HT + i, c0:c1]
            )
            nc.vector.tensor_mul(d[:, 0:w], d[:, 0:w], sel[:, i, c0:c1])
            o = work.tile([P, BS], F32, tag="o", name="o")
            nc.vector.tensor_add(
                o[:, 0:w], combb[:, HT + i, c0:c1], d[:, 0:w]
            )
            nc.sync.dma_start(out[i, :, c0:c1], o[:, 0:w])

        for i in range(HT):
            wb = wpool.tile([P, CT, P], BF16, tag="wb", name="wbp")
            nc.sync.dma_start(wb[:], Wp[i])
            halves = (
                ((0, BS),) if i < HT - 1
                else ((0, BS // 2), (BS // 2, BS))
            )
            for c0, c1 in halves:
                ps = pspool.tile([P, BS], F32, tag="ps", name="ps")
                for n in range(CT):
                    rhs = (
                        combb[:, n, c0:c1] if n < HT
                        else newc[:, n - HT, c0:c1]
                    )
                    nc.tensor.matmul(
                        ps[:, 0:c1 - c0], wb[:, n:n + 1, :], rhs,
                        start=(n == 0), stop=(n == CT - 1),
                    )
                pchain(i, c0, c1, ps)

    nc.finalize()
    return nc


def _get_program():
    global _PROGRAM
    if _PROGRAM is None:
        _PROGRAM = _build_program()
    return _PROGRAM


def _pack_weight(w, scale, npdtype):
    """[H, C] fp32 -> [HT, P, CT, P] with [i, p, n, m] = W[i*128+m, n*128+p].

    Slice [i, :, n, :] is the stationary operand (lhsT = W.T tile) for
    contraction tile n of output tile i.
    """
    wb = (np.asarray(w, dtype=np.float32) * scale).astype(npdtype)
    return np.ascontiguousarray(wb.reshape(HT, P, CT, P).transpose(0, 3, 2, 1))


def _prep_inputs(x, h, W_update, b_update, W_select, b_select, W_predict,
                 b_predict):
    x = np.asarray(x, dtype=np.float32)
    h = np.asarray(h, dtype=np.float32)
    comb = np.concatenate([x, h], axis=1)

    Wu8 = _pack_weight(W_update, WSCALE, NPF8)
    Ws = _pack_weight(W_select, 1.0, NPBF16)
    Wp = _pack_weight(W_predict, 1.0, NPBF16)
    bias = np.ascontiguousarray(
        np.concatenate(
            [
                np.asarray(b_update, dtype=np.float32).reshape(HT, P).T,
                np.asarray(b_select, dtype=np.float32).reshape(HT, P).T,
                np.asarray(b_predict, dtype=np.float32).reshape(HT, P).T,
            ],
            axis=1,
        )
    )

    in_maps = []
    for c in range(NCORES):
        rows = slice(c * BS, (c + 1) * BS)
        ct = np.ascontiguousarray(
            comb[rows].T.reshape(CT, P, BS).transpose(1, 0, 2)
        )
        in_maps.append(
            {
                "xh8": ct.astype(NPF8),
                "xhb": ct.astype(NPBF16),
                "Wu8": Wu8,
                "Ws": Ws,
                "Wp": Wp,
                "bias": bias,
            }
        )
    return in_maps


def kernel(x, h, W_update, b_update, W_select, b_select, W_predict, b_predict,
           _trace=False):
    nc = _get_program()
    in_maps = _prep_inputs(
        x, h, W_update, b_update, W_select, b_select, W_predict, b_predict
    )
    res = run_bass_kernel_spmd(
        nc, in_maps, core_ids=list(range(NCORES)), trace=_trace
    )
    h_new = np.empty((B, H), dtype=np.float32)
    for c in range(NCORES):
        rows = slice(c * BS, (c + 1) * BS)
        h_new[rows] = res.results[c]["out"].reshape(H, BS).T
    if _trace:
        return h_new, res
    return h_new
